# revision 17
# baseline (speedup 1.0000x reference)
"""Trainium2 Bass kernel for a CAM (channel-attention) module.

Reference computation (per batch b):
    v    = x[b].reshape(C, H*W)                  # C x N
    e    = v @ v.T                               # C x C Gram matrix
    attn = softmax(rowmax(e) - e, axis=-1)       # == exp(rowmin(e)-e) / rowsum
    out  = gamma * (attn @ v) + x[b]

Sharding: data-parallel over batch B=16 across 8 NeuronCores (2 batches/core,
no cross-core communication).

v5 design (PE-stream minimization + HAM management + short tail chains):
  - bf16 I/O (host cast), DVE-cast fp8 v8q, fp8 DoubleRow matmuls.
  - energy DR matmuls interleaved into the transpose phase at half-group
    granularity, accumulating into 4 per-row PSUM banks; the last two
    halves are emitted per-row so rows finish staggered (row m's softmax
    chain starts while rows m-1..0 still stream matmuls).
  - softmax chain is just rowmin -> exp: the gamma/Z normalization is
    folded into the out eviction (DVE scalar_tensor_tensor:
    o = psum * (gamma/Z_row) + x), which also carries the residual.
    ut transposes read the raw exp output u.
  - ut transposes reuse the transpose PSUM pool: 2 (pst) + 4 (energy
    rows) + 2 (out) = 8 PSUM banks exactly.
  - two-batch software pipeline: b1's transpose groups + wide dummy
    matmuls fill b0's tail bubbles so the HAM MID window never
    re-throttles the PE clock; b1's v8 casts are emitted late and in
    512-col pieces so the DVE FIFO never blocks softmax chains.
"""

import numpy as np

P = 128
C = 512
N = 4096
CT = C // P      # 4 c-tiles
NT = N // P      # 32 n-tiles
CH = 512         # chunk width (matmul free dim)
NCH = N // CH    # 8 n-chunks / vT groups
QN = N // 4      # 1024 quarter width
B = 16
NCORES = 8
BPC = B // NCORES  # batches per core

_CACHE = {}


def _build_program():
    import concourse.bacc as bacc
    import concourse.mybir as mybir
    import concourse.tile as tile
    from concourse.masks import make_identity

    f32 = mybir.dt.float32
    bf16 = mybir.dt.bfloat16
    f8 = mybir.dt.float8e4
    Alu = mybir.AluOpType
    Act = mybir.ActivationFunctionType
    DR = mybir.MatmulPerfMode.DoubleRow

    nc = bacc.Bacc("TRN2", target_bir_lowering=False, debug=False)
    x_d = nc.dram_tensor("x", [BPC, C, N], bf16, kind="ExternalInput").ap()
    g_d = nc.dram_tensor("gamma", [1], f32, kind="ExternalInput").ap()
    o_d = nc.dram_tensor("out", [BPC, C, N], bf16, kind="ExternalOutput").ap()

    with tile.TileContext(nc) as tc:
        with (
            tc.tile_pool(name="const", bufs=1) as const_pool,
            tc.tile_pool(name="xbp", bufs=2) as xb_pool,
            tc.tile_pool(name="v8p", bufs=2) as v8_pool,
            tc.tile_pool(name="vtp", bufs=2) as vt_pool,
            tc.tile_pool(name="up", bufs=2) as u_pool,
            tc.tile_pool(name="stat", bufs=2) as st_pool,
            tc.tile_pool(name="outp", bufs=6) as out_pool,
            tc.tile_pool(name="pst", bufs=2, space="PSUM") as ps_t_pool,
            tc.tile_pool(name="pse", bufs=1, space="PSUM") as ps_e_pool,
            tc.tile_pool(name="pso", bufs=2, space="PSUM") as ps_o_pool,
        ):
            ident_h = const_pool.tile([P, P], bf16, tag="identh")
            make_identity(nc, ident_h)
            ident_f = const_pool.tile([P, P], f32, tag="identf")
            make_identity(nc, ident_f)
            gamma_bc = const_pool.tile([P, 1], f32, tag="gamma")
            wide_id = const_pool.tile([P, CH], bf16, tag="wideid")
            nc.vector.memset(wide_id, 0.0)

            warm = ps_o_pool.tile([P, CH], f32, tag="pso", name="ham_warm")

            def keep_warm(n):
                # 512-wide dummy matmuls: sustained matmul-mode activity
                # for the HAM MID window (transpose-mode reads as idle;
                # one idle window halves the PE clock).
                for w in range(n):
                    nc.tensor.matmul(warm, ident_h, wide_id,
                                     start=(w == 0), stop=(w == n - 1))

            class Batch:
                def __init__(self, b):
                    self.b = b
                    self.xb = [[None] * 4 for _ in range(CT)]
                    self.v8q = [None] * 4
                    self.vT = [None] * NCH
                    for q in range(4):
                        self.v8q[q] = v8_pool.tile(
                            [P, CT, QN], f8, tag=f"v8q{q}", name=f"v8_{b}_{q}")
                    self.pse = [ps_e_pool.tile([P, C], f32, tag=f"row{m}",
                                               name=f"pse_{b}_{m}")
                                for m in range(CT)]
                    self.mins = st_pool.tile([P, CT], f32, tag="mins",
                                             name=f"mins_{b}")
                    self.zsum = st_pool.tile([P, CT], f32, tag="zsum",
                                             name=f"zsum_{b}")
                    self.gz = st_pool.tile([P, CT], f32, tag="gz",
                                           name=f"gz_{b}")
                    self.u_sb = u_pool.tile([P, CT, C], bf16, tag="u",
                                            name=f"u_{b}")
                    self.ut_sb = u_pool.tile([P, CT, C], f8, tag="ut",
                                             name=f"ut_{b}")
                    self.e_sb = u_pool.tile([P, CT, 3 * P], f32, tag="esb",
                                            name=f"e_{b}")

                def dma_in(self):
                    b = self.b
                    for q in range(4):
                        for ct in range(CT):
                            t = xb_pool.tile([P, QN], bf16,
                                             tag=f"xb{ct}q{q}",
                                             name=f"xb_{b}_{ct}_{q}")
                            nc.sync.dma_start(
                                t, x_d[b, ct * P:(ct + 1) * P,
                                       q * QN:(q + 1) * QN])
                            self.xb[ct][q] = t

                def casts(self, qs, split=1):
                    # bf16 -> fp8 v8q casts; split>1 emits smaller pieces
                    # so a DMA-gated cast can never block the DVE FIFO
                    # for long at a phase boundary.
                    w = QN // split
                    for q in qs:
                        for ct in range(CT):
                            for s in range(split):
                                nc.vector.tensor_copy(
                                    self.v8q[q][:, ct, s * w:(s + 1) * w],
                                    self.xb[ct][q][:, s * w:(s + 1) * w])

                def group_half(self, g, k):
                    # 8 PE transposes of 128x128 bf16 blocks + fp8 eviction
                    if k == 0:
                        self.vT[g] = vt_pool.tile([P, 4, CH], f8,
                                                  tag=f"vt{g}",
                                                  name=f"vT_{self.b}_{g}")
                    ps = ps_t_pool.tile([P, 2, CH], bf16, tag="pst",
                                        name=f"ps_t_{self.b}_{g}_{k}")
                    for s2 in range(2):
                        nt = 4 * g + 2 * k + s2
                        q, loc = nt // 8, nt % 8
                        for cb in range(CT):
                            nc.tensor.transpose(
                                ps[:, s2, cb * P:(cb + 1) * P],
                                self.xb[cb][q][:, loc * P:(loc + 1) * P],
                                ident_h)
                    nc.scalar.activation(self.vT[g][:, 2 * k:2 * k + 2, :],
                                         ps, Act.Copy)

                def energy_half(self, i, rows):
                    # energy DR matmuls for half-group i: descending
                    # lower-triangle; row m covers cols [0,(m+1)*128)
                    gp, k = i // 2, i % 2
                    for m in rows:
                        W = (m + 1) * P
                        nc.tensor.matmul(
                            self.pse[m][:, :W],
                            self.vT[gp][:, 2 * k:2 * k + 2,
                                        m * P:(m + 1) * P],
                            self.vT[gp][:, 2 * k:2 * k + 2, :W],
                            start=(i == 0),
                            stop=(i == 2 * NCH - 1),
                            perf_mode=DR)

                def sm(self, m):
                    # softmax row m: rowmin -> exp(+rowsum); gamma/Z is
                    # applied at out eviction. PSUM bank m free after exp.
                    nc.vector.tensor_reduce(
                        self.mins[:, m:m + 1], self.pse[m],
                        axis=mybir.AxisListType.X, op=Alu.min)
                    if m > 0:
                        # copy the block the next upper() needs first
                        j = m - 1
                        nc.scalar.activation(
                            self.e_sb[:, m, j * P:m * P],
                            self.pse[m][:, j * P:m * P], Act.Copy)
                        if m > 1:
                            nc.scalar.activation(
                                self.e_sb[:, m, :j * P],
                                self.pse[m][:, :j * P], Act.Copy)
                    nc.scalar.activation(
                        self.u_sb[:, m, :], self.pse[m], Act.Exp,
                        bias=self.mins[:, m:m + 1], scale=-1.0,
                        accum_out=self.zsum[:, m:m + 1])
                    nc.vector.reciprocal(self.gz[:, m:m + 1],
                                         self.zsum[:, m:m + 1])
                    nc.vector.tensor_tensor(
                        self.gz[:, m:m + 1], self.gz[:, m:m + 1], gamma_bc,
                        Alu.mult)

                def upper(self, m):
                    # complete row m cols >= (m+1)*128 by Gram symmetry
                    for j in range(m + 1, CT):
                        nc.tensor.transpose(
                            self.pse[m][:, j * P:(j + 1) * P],
                            self.e_sb[:, j, m * P:(m + 1) * P],
                            ident_f)

                def utT(self, m):
                    # PE-transpose u row m -> ut_sb[d, kt, m-cols] fp8
                    pu = ps_t_pool.tile([P, 2, CH], bf16, tag="pst",
                                        name=f"ps_u_{self.b}_{m}")
                    for kt in range(CT):
                        nc.tensor.transpose(
                            pu[:, kt // 2, (kt % 2) * P:(kt % 2 + 1) * P],
                            self.u_sb[:, m, kt * P:(kt + 1) * P],
                            ident_h)
                    for t in range(2):
                        nc.scalar.activation(
                            self.ut_sb[:, 2 * t:2 * t + 2,
                                       m * P:(m + 1) * P],
                            pu[:, t, :2 * P], Act.Copy)

                def out(self, m):
                    # out row m: DR matmuls; eviction scales by gamma/Z
                    # and adds the residual in one DVE op.
                    b = self.b
                    for pair in range(4):
                        o = out_pool.tile([P, QN], bf16, tag="o",
                                          name=f"o_{b}_{m}_{pair}")
                        for h in range(2):
                            ps = ps_o_pool.tile(
                                [P, CH], f32, tag="pso",
                                name=f"ps_o_{b}_{m}_{pair}_{h}")
                            for t in range(2):
                                nc.tensor.matmul(
                                    ps,
                                    self.ut_sb[:, 2 * t:2 * t + 2,
                                               m * P:(m + 1) * P],
                                    self.v8q[pair][:, 2 * t:2 * t + 2,
                                                   h * CH:(h + 1) * CH],
                                    start=(t == 0), stop=(t == 1),
                                    perf_mode=DR)
                            nc.vector.scalar_tensor_tensor(
                                o[:, h * CH:(h + 1) * CH], ps,
                                self.gz[:, m:m + 1],
                                self.xb[m][pair][:, h * CH:(h + 1) * CH],
                                op0=Alu.mult, op1=Alu.add)
                        nc.sync.dma_start(
                            o_d[b, m * P:(m + 1) * P,
                                pair * QN:(pair + 1) * QN],
                            o)

            ALL = list(range(CT - 1, -1, -1))
            H = 2 * NCH  # 16 half-groups
            b0 = Batch(0)
            b1 = Batch(1)

            # ---- b0 loads + casts up front (DVE is otherwise empty) ------
            b0.dma_in()
            b0.casts(range(4))
            nc.sync.dma_start(gamma_bc, g_d.to_broadcast((P, 1)))

            # HAM warm-up burn while the first DMAs land
            for w in range(32):
                nc.tensor.matmul(warm[:, :P], ident_h, ident_h,
                                 start=(w == 0), stop=(w == 31))

            # ---- b0 groups: transposes with half-lagged energy; the two
            # last halves are deferred to the per-row cascade below.
            for g in range(NCH):
                for k in range(2):
                    i = 2 * g + k
                    b0.group_half(g, k)
                    if i < 3:
                        keep_warm(2)
                    if 1 <= i <= H - 2:
                        b0.energy_half(i - 1, ALL)

            # b1 input DMAs start flowing during b0's tail
            b1.dma_in()

            # ---- b0 tail cascade interleaved with b1 early groups --------
            b0.energy_half(H - 2, [3])
            b0.energy_half(H - 1, [3])
            b0.sm(3)
            keep_warm(4)
            b1.group_half(0, 0)
            b1.group_half(0, 1)
            b0.utT(3)
            b0.energy_half(H - 2, [2])
            b0.energy_half(H - 1, [2])
            b0.upper(2)
            b0.sm(2)
            b0.out(3)
            b1.group_half(1, 0)
            b1.group_half(1, 1)
            b0.utT(2)
            b0.energy_half(H - 2, [1])
            b0.energy_half(H - 1, [1])
            b0.upper(1)
            b0.sm(1)
            b0.out(2)
            b1.group_half(2, 0)
            b1.group_half(2, 1)
            b0.utT(1)
            b0.energy_half(H - 2, [0])
            b0.energy_half(H - 1, [0])
            b0.upper(0)
            b0.sm(0)
            b0.out(1)
            for i in range(4):      # b1 energy catch-up (pse banks now free)
                b1.energy_half(i, ALL)
            b0.utT(0)
            b0.out(0)

            # b1 casts emitted only now, in 512-col pieces: DVE FIFO stays
            # clear for b0's softmax chains + residual evictions
            b1.casts(range(4), split=2)

            # ---- b1 late groups with 2-half-lagged energy ----------------
            for g in range(3, NCH):
                for k in range(2):
                    i = 2 * g + k
                    b1.group_half(g, k)
                    if i - 2 <= H - 3:
                        b1.energy_half(i - 2, ALL)

            # ---- b1 tail cascade (keep_warm bridges the bubbles) ---------
            b1.energy_half(H - 2, [3])
            b1.energy_half(H - 1, [3])
            b1.sm(3)
            keep_warm(6)
            b1.utT(3)
            b1.energy_half(H - 2, [2])
            b1.energy_half(H - 1, [2])
            b1.upper(2)
            b1.sm(2)
            b1.out(3)
            b1.utT(2)
            b1.energy_half(H - 2, [1])
            b1.energy_half(H - 1, [1])
            b1.upper(1)
            b1.sm(1)
            b1.out(2)
            b1.utT(1)
            b1.energy_half(H - 2, [0])
            b1.energy_half(H - 1, [0])
            b1.upper(0)
            b1.sm(0)
            b1.out(1)
            b1.utT(0)
            b1.out(0)

    nc.compile()
    return nc


def _get_program():
    if "nc" not in _CACHE:
        _CACHE["nc"] = _build_program()
    return _CACHE["nc"]


def kernel(x: np.ndarray, gamma: np.ndarray) -> np.ndarray:
    import ml_dtypes
    from concourse.bass_utils import run_bass_kernel_spmd

    assert x.shape == (B, C, 64, 64), x.shape
    bf = ml_dtypes.bfloat16
    # bf16 on-device pipeline: rel err ~2^-9, well within the 2e-2 gate
    xh = np.ascontiguousarray(x, dtype=np.float32).astype(bf)
    gamma = np.ascontiguousarray(gamma, dtype=np.float32).reshape(1)

    nc = _get_program()
    xs = xh.reshape(NCORES, BPC, C, N)
    in_maps = [{"x": xs[i], "gamma": gamma} for i in range(NCORES)]
    res = run_bass_kernel_spmd(nc, in_maps, list(range(NCORES)))
    out = np.empty((NCORES, BPC, C, N), dtype=np.float32)
    for i in range(NCORES):
        out[i] = res.results[i]["out"].astype(np.float32)
    return out.reshape(B, C, 64, 64)


# revision 18
# speedup vs baseline: 1.2292x; 1.2292x over previous
"""Trainium2 Bass kernel for a CAM (channel-attention) module.

Reference computation (per batch b):
    v    = x[b].reshape(C, H*W)                  # C x N
    e    = v @ v.T                               # C x C Gram matrix
    attn = softmax(rowmax(e) - e, axis=-1)       # == exp(rowmin(e)-e) / rowsum
    out  = gamma * (attn @ v) + x[b]

Sharding: data-parallel over batch B=16 across 8 NeuronCores (2 batches/core,
no cross-core communication).

v5 design (PE-stream minimization + HAM management + short tail chains):
  - bf16 I/O (host cast), DVE-cast fp8 v8q, fp8 DoubleRow matmuls.
  - energy DR matmuls interleaved into the transpose phase at half-group
    granularity, accumulating into 4 per-row PSUM banks; the last two
    halves are emitted per-row so rows finish staggered (row m's softmax
    chain starts while rows m-1..0 still stream matmuls).
  - softmax chain is just rowmin -> exp: the gamma/Z normalization is
    folded into the out eviction (DVE scalar_tensor_tensor:
    o = psum * (gamma/Z_row) + x), which also carries the residual.
    ut transposes read the raw exp output u.
  - ut transposes reuse the transpose PSUM pool: 2 (pst) + 4 (energy
    rows) + 2 (out) = 8 PSUM banks exactly.
  - two-batch software pipeline: b1's transpose groups + wide dummy
    matmuls fill b0's tail bubbles so the HAM MID window never
    re-throttles the PE clock; b1's v8 casts are emitted late and in
    512-col pieces so the DVE FIFO never blocks softmax chains.
"""

import numpy as np

P = 128
C = 512
N = 4096
CT = C // P      # 4 c-tiles
NT = N // P      # 32 n-tiles
CH = 512         # chunk width (matmul free dim)
NCH = N // CH    # 8 n-chunks / vT groups
QN = N // 4      # 1024 quarter width
B = 16
NCORES = 8
BPC = B // NCORES  # batches per core

_CACHE = {}


def _build_program():
    import concourse.bacc as bacc
    import concourse.mybir as mybir
    import concourse.tile as tile
    from concourse.masks import make_identity

    f32 = mybir.dt.float32
    bf16 = mybir.dt.bfloat16
    f8 = mybir.dt.float8e4
    Alu = mybir.AluOpType
    Act = mybir.ActivationFunctionType
    DR = mybir.MatmulPerfMode.DoubleRow

    nc = bacc.Bacc("TRN2", target_bir_lowering=False, debug=False)
    x_d = nc.dram_tensor("x", [BPC, C, N], bf16, kind="ExternalInput").ap()
    g_d = nc.dram_tensor("gamma", [1], f32, kind="ExternalInput").ap()
    o_d = nc.dram_tensor("out", [BPC, C, N], bf16, kind="ExternalOutput").ap()

    with tile.TileContext(nc) as tc:
        with (
            tc.tile_pool(name="const", bufs=1) as const_pool,
            tc.tile_pool(name="xbp", bufs=2) as xb_pool,
            tc.tile_pool(name="v8p", bufs=2) as v8_pool,
            tc.tile_pool(name="vtp", bufs=2) as vt_pool,
            tc.tile_pool(name="up", bufs=2) as u_pool,
            tc.tile_pool(name="stat", bufs=2) as st_pool,
            tc.tile_pool(name="outp", bufs=6) as out_pool,
            tc.tile_pool(name="pst", bufs=2, space="PSUM") as ps_t_pool,
            tc.tile_pool(name="pse", bufs=1, space="PSUM") as ps_e_pool,
            tc.tile_pool(name="pso", bufs=2, space="PSUM") as ps_o_pool,
        ):
            ident_h = const_pool.tile([P, P], bf16, tag="identh")
            make_identity(nc, ident_h)
            ident_f = const_pool.tile([P, P], f32, tag="identf")
            make_identity(nc, ident_f)
            gamma_bc = const_pool.tile([P, 1], f32, tag="gamma")
            wide_id = const_pool.tile([P, CH], bf16, tag="wideid")
            nc.vector.memset(wide_id, 0.0)

            warm = ps_o_pool.tile([P, CH], f32, tag="pso", name="ham_warm")

            def keep_warm(n):
                # 512-wide dummy matmuls: sustained matmul-mode activity
                # for the HAM MID window (transpose-mode reads as idle;
                # one idle window halves the PE clock).
                for w in range(n):
                    nc.tensor.matmul(warm, ident_h, wide_id,
                                     start=(w == 0), stop=(w == n - 1))

            class Batch:
                def __init__(self, b):
                    self.b = b
                    self.xb = [[None] * 4 for _ in range(CT)]
                    self.v8q = [None] * 4
                    self.vT = [None] * NCH
                    for q in range(4):
                        self.v8q[q] = v8_pool.tile(
                            [P, CT, QN], f8, tag=f"v8q{q}", name=f"v8_{b}_{q}")
                    self.pse = [ps_e_pool.tile([P, C], f32, tag=f"row{m}",
                                               name=f"pse_{b}_{m}")
                                for m in range(CT)]
                    self.mins = st_pool.tile([P, CT], f32, tag="mins",
                                             name=f"mins_{b}")
                    self.zsum = st_pool.tile([P, CT], f32, tag="zsum",
                                             name=f"zsum_{b}")
                    self.gz = st_pool.tile([P, CT], f32, tag="gz",
                                           name=f"gz_{b}")
                    self.u_sb = u_pool.tile([P, CT, C], bf16, tag="u",
                                            name=f"u_{b}")
                    self.ut_sb = u_pool.tile([P, CT, C], f8, tag="ut",
                                             name=f"ut_{b}")
                    self.e_sb = u_pool.tile([P, CT, 3 * P], f32, tag="esb",
                                            name=f"e_{b}")

                def dma_in(self):
                    b = self.b
                    for q in range(4):
                        for ct in range(CT):
                            t = xb_pool.tile([P, QN], bf16,
                                             tag=f"xb{ct}q{q}",
                                             name=f"xb_{b}_{ct}_{q}")
                            nc.sync.dma_start(
                                t, x_d[b, ct * P:(ct + 1) * P,
                                       q * QN:(q + 1) * QN])
                            self.xb[ct][q] = t

                def casts(self, qs, split=1):
                    # bf16 -> fp8 v8q casts; split>1 emits smaller pieces
                    # so a DMA-gated cast can never block the DVE FIFO
                    # for long at a phase boundary.
                    w = QN // split
                    for q in qs:
                        for ct in range(CT):
                            for s in range(split):
                                nc.vector.tensor_copy(
                                    self.v8q[q][:, ct, s * w:(s + 1) * w],
                                    self.xb[ct][q][:, s * w:(s + 1) * w])

                def group_half(self, g, k):
                    # 8 PE transposes of 128x128 bf16 blocks + fp8 eviction
                    if k == 0:
                        self.vT[g] = vt_pool.tile([P, 4, CH], f8,
                                                  tag=f"vt{g}",
                                                  name=f"vT_{self.b}_{g}")
                    ps = ps_t_pool.tile([P, 2, CH], bf16, tag="pst",
                                        name=f"ps_t_{self.b}_{g}_{k}")
                    for s2 in range(2):
                        nt = 4 * g + 2 * k + s2
                        q, loc = nt // 8, nt % 8
                        for cb in range(CT):
                            nc.tensor.transpose(
                                ps[:, s2, cb * P:(cb + 1) * P],
                                self.xb[cb][q][:, loc * P:(loc + 1) * P],
                                ident_h)
                    nc.scalar.activation(self.vT[g][:, 2 * k:2 * k + 2, :],
                                         ps, Act.Copy)

                def energy_half(self, i, rows):
                    # energy DR matmuls for half-group i: descending
                    # lower-triangle; row m covers cols [0,(m+1)*128)
                    gp, k = i // 2, i % 2
                    for m in rows:
                        W = (m + 1) * P
                        nc.tensor.matmul(
                            self.pse[m][:, :W],
                            self.vT[gp][:, 2 * k:2 * k + 2,
                                        m * P:(m + 1) * P],
                            self.vT[gp][:, 2 * k:2 * k + 2, :W],
                            start=(i == 0),
                            stop=(i == 2 * NCH - 1),
                            perf_mode=DR)

                def sm(self, m):
                    # softmax row m: rowmin -> exp(+rowsum); gamma/Z is
                    # applied at out eviction. PSUM bank m free after exp.
                    nc.vector.tensor_reduce(
                        self.mins[:, m:m + 1], self.pse[m],
                        axis=mybir.AxisListType.X, op=Alu.min)
                    if m > 0:
                        # copy the block the next upper() needs first
                        j = m - 1
                        nc.scalar.activation(
                            self.e_sb[:, m, j * P:m * P],
                            self.pse[m][:, j * P:m * P], Act.Copy)
                        if m > 1:
                            nc.scalar.activation(
                                self.e_sb[:, m, :j * P],
                                self.pse[m][:, :j * P], Act.Copy)
                    nc.scalar.activation(
                        self.u_sb[:, m, :], self.pse[m], Act.Exp,
                        bias=self.mins[:, m:m + 1], scale=-1.0,
                        accum_out=self.zsum[:, m:m + 1])
                    nc.vector.reciprocal(self.gz[:, m:m + 1],
                                         self.zsum[:, m:m + 1])
                    nc.vector.tensor_tensor(
                        self.gz[:, m:m + 1], self.gz[:, m:m + 1], gamma_bc,
                        Alu.mult)

                def upper(self, m):
                    # complete row m cols >= (m+1)*128 by Gram symmetry
                    for j in range(m + 1, CT):
                        nc.tensor.transpose(
                            self.pse[m][:, j * P:(j + 1) * P],
                            self.e_sb[:, j, m * P:(m + 1) * P],
                            ident_f)

                def utT(self, m):
                    # PE-transpose u row m -> ut_sb[d, kt, m-cols] fp8
                    pu = ps_t_pool.tile([P, 2, CH], bf16, tag="pst",
                                        name=f"ps_u_{self.b}_{m}")
                    for kt in range(CT):
                        nc.tensor.transpose(
                            pu[:, kt // 2, (kt % 2) * P:(kt % 2 + 1) * P],
                            self.u_sb[:, m, kt * P:(kt + 1) * P],
                            ident_h)
                    for t in range(2):
                        nc.scalar.activation(
                            self.ut_sb[:, 2 * t:2 * t + 2,
                                       m * P:(m + 1) * P],
                            pu[:, t, :2 * P], Act.Copy)

                def out(self, m, drain=False):
                    # out row m: DR matmuls; eviction scales by gamma/Z
                    # and adds the residual in one DVE op. drain=True
                    # (kernel-final rows): store each 512-chunk as soon
                    # as its eviction lands instead of waiting for the
                    # full 1024-col pair, shortening the end-of-kernel
                    # eviction+store drain.
                    b = self.b
                    for pair in range(4):
                        o = out_pool.tile([P, QN], bf16, tag="o",
                                          name=f"o_{b}_{m}_{pair}")
                        for h in range(2):
                            ps = ps_o_pool.tile(
                                [P, CH], f32, tag="pso",
                                name=f"ps_o_{b}_{m}_{pair}_{h}")
                            for t in range(2):
                                nc.tensor.matmul(
                                    ps,
                                    self.ut_sb[:, 2 * t:2 * t + 2,
                                               m * P:(m + 1) * P],
                                    self.v8q[pair][:, 2 * t:2 * t + 2,
                                                   h * CH:(h + 1) * CH],
                                    start=(t == 0), stop=(t == 1),
                                    perf_mode=DR)
                            nc.vector.scalar_tensor_tensor(
                                o[:, h * CH:(h + 1) * CH], ps,
                                self.gz[:, m:m + 1],
                                self.xb[m][pair][:, h * CH:(h + 1) * CH],
                                op0=Alu.mult, op1=Alu.add)
                            if drain:
                                nc.sync.dma_start(
                                    o_d[b, m * P:(m + 1) * P,
                                        pair * QN + h * CH:
                                        pair * QN + (h + 1) * CH],
                                    o[:, h * CH:(h + 1) * CH])
                        if not drain:
                            nc.sync.dma_start(
                                o_d[b, m * P:(m + 1) * P,
                                    pair * QN:(pair + 1) * QN],
                                o)

            ALL = list(range(CT - 1, -1, -1))
            H = 2 * NCH  # 16 half-groups
            b0 = Batch(0)
            b1 = Batch(1)

            # ---- b0 loads + casts up front (DVE is otherwise empty) ------
            b0.dma_in()
            b0.casts(range(4))
            nc.sync.dma_start(gamma_bc, g_d.to_broadcast((P, 1)))

            # HAM warm-up burn while the first DMAs land
            for w in range(32):
                nc.tensor.matmul(warm[:, :P], ident_h, ident_h,
                                 start=(w == 0), stop=(w == 31))

            # ---- b0 groups: transposes with half-lagged energy; the two
            # last halves are deferred to the per-row cascade below.
            for g in range(NCH):
                for k in range(2):
                    i = 2 * g + k
                    b0.group_half(g, k)
                    if i < 3:
                        keep_warm(2)
                    if 1 <= i <= H - 2:
                        b0.energy_half(i - 1, ALL)

            # b1 input DMAs start flowing during b0's tail
            b1.dma_in()

            # ---- b0 tail cascade interleaved with b1 early groups --------
            b0.energy_half(H - 2, [3])
            b0.energy_half(H - 1, [3])
            b0.sm(3)
            keep_warm(4)
            b1.group_half(0, 0)
            b1.group_half(0, 1)
            b0.utT(3)
            b0.energy_half(H - 2, [2])
            b0.energy_half(H - 1, [2])
            b0.upper(2)
            b0.sm(2)
            b0.out(3)
            b1.group_half(1, 0)
            b1.group_half(1, 1)
            b0.utT(2)
            b0.energy_half(H - 2, [1])
            b0.energy_half(H - 1, [1])
            b0.upper(1)
            b0.sm(1)
            b0.out(2)
            b1.group_half(2, 0)
            b1.group_half(2, 1)
            b0.utT(1)
            b0.energy_half(H - 2, [0])
            b0.energy_half(H - 1, [0])
            b0.upper(0)
            b0.sm(0)
            b0.out(1)
            for i in range(4):      # b1 energy catch-up (pse banks now free)
                b1.energy_half(i, ALL)
            b0.utT(0)
            b0.out(0)

            # b1 casts emitted only now, in 512-col pieces: DVE FIFO stays
            # clear for b0's softmax chains + residual evictions
            b1.casts(range(4), split=2)

            # ---- b1 late groups with 2-half-lagged energy ----------------
            for g in range(3, NCH):
                for k in range(2):
                    i = 2 * g + k
                    b1.group_half(g, k)
                    if i - 2 <= H - 3:
                        b1.energy_half(i - 2, ALL)

            # ---- b1 tail cascade (keep_warm bridges the bubbles) ---------
            b1.energy_half(H - 2, [3])
            b1.energy_half(H - 1, [3])
            b1.sm(3)
            keep_warm(6)
            b1.utT(3)
            b1.energy_half(H - 2, [2])
            b1.energy_half(H - 1, [2])
            b1.upper(2)
            b1.sm(2)
            b1.out(3)
            b1.utT(2)
            b1.energy_half(H - 2, [1])
            b1.energy_half(H - 1, [1])
            b1.upper(1)
            b1.sm(1)
            b1.out(2)
            b1.utT(1)
            b1.energy_half(H - 2, [0])
            b1.energy_half(H - 1, [0])
            b1.upper(0)
            b1.sm(0)
            b1.out(1, drain=True)
            b1.utT(0)
            b1.out(0, drain=True)

    nc.compile()
    return nc


def _get_program():
    if "nc" not in _CACHE:
        _CACHE["nc"] = _build_program()
    return _CACHE["nc"]


def kernel(x: np.ndarray, gamma: np.ndarray) -> np.ndarray:
    import ml_dtypes
    from concourse.bass_utils import run_bass_kernel_spmd

    assert x.shape == (B, C, 64, 64), x.shape
    bf = ml_dtypes.bfloat16
    # bf16 on-device pipeline: rel err ~2^-9, well within the 2e-2 gate
    xh = np.ascontiguousarray(x, dtype=np.float32).astype(bf)
    gamma = np.ascontiguousarray(gamma, dtype=np.float32).reshape(1)

    nc = _get_program()
    xs = xh.reshape(NCORES, BPC, C, N)
    in_maps = [{"x": xs[i], "gamma": gamma} for i in range(NCORES)]
    res = run_bass_kernel_spmd(nc, in_maps, list(range(NCORES)))
    out = np.empty((NCORES, BPC, C, N), dtype=np.float32)
    for i in range(NCORES):
        out[i] = res.results[i]["out"].astype(np.float32)
    return out.reshape(B, C, 64, 64)
